# revision 4
# baseline (speedup 1.0000x reference)
"""Trainium2 Bass kernel for nn_DiffAttn (8-head euclidean-distance attention).

Sharding over 8 NeuronCores: core c = (head-group g = c % 2, batch b = c // 2).
Each core projects its batch's 512 tokens through its 4 heads' weight slices
(Q/K/V column slices), computes 4 attention units (one per head) and writes
a dense [512, 256] output slice plus a dense [4, 512, 512] score slice.

Per-core math (feature-major activations, D = 64 per head):
  QT/KT/VT  = relu(W_slice.T @ xT + b)        (PE + ACT, [256, 512] as 2 tiles)
  psum      = QT_h.T @ KT_h - kk/2            (K=64 matmul + rank-1 ones x kk)
  s_pos     = sqrt(psum * (-1/32) + qq/64)    (ACT, = sqrt(d2)/8 = -score)
  score_out = s_pos * (-qm)                   (DVE per-partition scalar)
  e         = exp(-s_pos), denom = rowsum(e)  (ACT with accum_out)
  outT      = sum_i V_nat_i.T-slice @ eT_i    (PE transposes of e, then A.V)
  out       = transpose(outT) * (qm / denom)  (PE transpose + DVE scale)

ACT-table discipline: all sqrts are emitted before all exps (different
activation table sets; each switch costs a 1283ns table reload).
"""
import sys
sys.path.insert(0, '/opt/trn_rl_repo')

import numpy as np
from contextlib import ExitStack

import concourse.bass as bass
import concourse.bacc as bacc
import concourse.tile as tile
from concourse import mybir, masks
from concourse.bass_utils import run_bass_kernel_spmd

F32 = mybir.dt.float32
H, B, L, U = 8, 4, 512, 512
D = U // H              # 64
N_CORES = 8
HPC = 4                 # heads per core
NG = H // HPC           # head groups = 2
W = HPC * D             # per-core output width = 256
NJ = L // 128           # 4 lq tiles per unit


def emit(ctx: ExitStack, tc, xq, xk, xv, wq, wk, wv, aux, out_d, score_d):
    nc = tc.nc

    aux_p = ctx.enter_context(tc.tile_pool(name="aux", bufs=1))
    const_p = ctx.enter_context(tc.tile_pool(name="const", bufs=1))
    xw_p = ctx.enter_context(tc.tile_pool(name="xw", bufs=2))
    qkv_p = ctx.enter_context(tc.tile_pool(name="qkv", bufs=2))
    vnat_p = ctx.enter_context(tc.tile_pool(name="vnat", bufs=4))
    sq_p = ctx.enter_context(tc.tile_pool(name="sq", bufs=4))
    rows_p = ctx.enter_context(tc.tile_pool(name="rows", bufs=5))
    cols_p = ctx.enter_context(tc.tile_pool(name="cols", bufs=4))
    spos_p = ctx.enter_context(tc.tile_pool(name="spos", bufs=16))
    score_p = ctx.enter_context(tc.tile_pool(name="score", bufs=2))
    e_p = ctx.enter_context(tc.tile_pool(name="e", bufs=8))
    etsb_p = ctx.enter_context(tc.tile_pool(name="etsb", bufs=8))
    den_p = ctx.enter_context(tc.tile_pool(name="den", bufs=4))
    avsb_p = ctx.enter_context(tc.tile_pool(name="avsb", bufs=2))
    outsb_p = ctx.enter_context(tc.tile_pool(name="outsb", bufs=1))
    psA = ctx.enter_context(tc.tile_pool(name="psA", bufs=4, space="PSUM"))
    psB = ctx.enter_context(tc.tile_pool(name="psB", bufs=3, space="PSUM"))

    Sqrt = mybir.ActivationFunctionType.Sqrt
    Exp = mybir.ActivationFunctionType.Exp
    Relu = mybir.ActivationFunctionType.Relu

    # --- constants and input staging ------------------------------------
    aux_t = aux_p.tile([128, 16], F32, tag="aux")
    nc.sync.dma_start(aux_t[:], aux)
    ident = const_p.tile([128, 128], F32, tag="ident")
    masks.make_identity(nc, ident[:])
    ones_t = const_p.tile([128, 128], F32, tag="ones")
    nc.gpsimd.memset(ones_t[:], 1.0)

    x_sb = {}
    w_sb = {}
    for name, xd, wd in (("q", xq, wq), ("k", xk, wk), ("v", xv, wv)):
        xt = xw_p.tile([128, 4 * L], F32, tag="x")
        nc.sync.dma_start(
            xt[:].rearrange("p (c l) -> p c l", c=4),
            xd.rearrange("(c p) l -> p c l", p=128),
        )
        x_sb[name] = xt
        wt = xw_p.tile([128, 4 * W], F32, tag="w")
        nc.sync.dma_start(
            wt[:].rearrange("p (c n) -> p c n", c=4),
            wd.rearrange("(c p) n -> p c n", p=128),
        )
        w_sb[name] = wt

    # --- projections -----------------------------------------------------
    # aux columns: 0-1 bq, 2-3 bk, 4-5 bv, 6-9 qm_neg, 10-13 qm_pos
    qt_tiles, kt_tiles, vt_tiles = [], [], []
    sq_tiles = {}
    for ti, (name, dest, boff) in enumerate(
        (("q", qt_tiles, 0), ("k", kt_tiles, 2), ("v", vt_tiles, 4))
    ):
        xt, wt = x_sb[name], w_sb[name]
        for m in range(2):
            pp = psA.tile([128, L], F32, tag="psA")
            for c in range(4):
                nc.tensor.matmul(
                    pp[:],
                    wt[:, c * W + m * 128 : c * W + (m + 1) * 128],
                    xt[:, c * L : (c + 1) * L],
                    start=(c == 0),
                    stop=(c == 3),
                )
            t = qkv_p.tile([128, L], F32, tag=name + "t")
            nc.scalar.activation(
                t[:], pp[:], Relu, bias=aux_t[:, boff + m : boff + m + 1], scale=1.0
            )
            dest.append(t)
            if name in ("q", "k"):
                sq = sq_p.tile([128, L], F32, tag="sq")
                nc.vector.tensor_mul(sq[:], t[:], t[:])
                sq_tiles[(name, m)] = sq

    # --- qq (column form, pre-scaled 1/64) and kk rows (pre-scaled -1/2) -
    qq_cols = []   # per local head: [128, NJ] sbuf, qq/64
    kk_rows = []   # per local head: [1, L] sbuf, -kk/2
    for hh in range(HPC):
        m, s = hh // 2, hh % 2
        # qq row = ones(64).T @ QT_h^2
        qrow_ps = psB.tile([1, L], F32, tag="psB")
        nc.tensor.matmul(
            qrow_ps[:],
            ones_t[s * 64 : (s + 1) * 64, 0:1],
            sq_tiles[("q", m)][s * 64 : (s + 1) * 64, :],
            start=True,
            stop=True,
        )
        qrow_sb = rows_p.tile([1, L], F32, tag="qrow")
        nc.vector.tensor_scalar_mul(qrow_sb[:], qrow_ps[:], 1.0 / 64.0)
        colT_ps = psB.tile([128, NJ], F32, tag="psB")
        for j in range(NJ):
            nc.tensor.transpose(
                colT_ps[:, j : j + 1],
                qrow_sb[0:1, j * 128 : (j + 1) * 128],
                ident[0:1, 0:1],
            )
        qq_col = cols_p.tile([128, NJ], F32, tag="qqcol")
        nc.vector.tensor_copy(qq_col[:], colT_ps[:])
        qq_cols.append(qq_col)

        krow_ps = psB.tile([1, L], F32, tag="psB")
        nc.tensor.matmul(
            krow_ps[:],
            ones_t[s * 64 : (s + 1) * 64, 0:1],
            sq_tiles[("k", m)][s * 64 : (s + 1) * 64, :],
            start=True,
            stop=True,
        )
        kk_row = rows_p.tile([1, L], F32, tag="kkrow")
        nc.vector.tensor_scalar_mul(kk_row[:], krow_ps[:], -0.5)
        kk_rows.append(kk_row)

    # --- V natural layout: vnat_i [128, W], i = token chunk --------------
    vnat = [vnat_p.tile([128, W], F32, tag="vnat", name=f"vnat{i}") for i in range(NJ)]
    for m in range(2):
        for i in range(NJ):
            vt_ps = psB.tile([128, 128], F32, tag="psB")
            nc.tensor.transpose(
                vt_ps[:], vt_tiles[m][:, i * 128 : (i + 1) * 128], ident[:]
            )
            nc.vector.tensor_copy(vnat[i][:, m * 128 : (m + 1) * 128], vt_ps[:])

    # --- distance + sqrt + score output per head -------------------------
    spos = {}
    for hh in range(HPC):
        m, s = hh // 2, hh % 2
        qt, kt = qt_tiles[m], kt_tiles[m]
        for j in range(NJ):
            d2_ps = psA.tile([128, L], F32, tag="psA")
            nc.tensor.matmul(
                d2_ps[:],
                qt[s * 64 : (s + 1) * 64, j * 128 : (j + 1) * 128],
                kt[s * 64 : (s + 1) * 64, :],
                start=True,
                stop=False,
            )
            nc.tensor.matmul(
                d2_ps[:],
                ones_t[0:1, 0:128],
                kk_rows[hh][0:1, :],
                start=False,
                stop=True,
            )
            sp = spos_p.tile([128, L], F32, tag="spos")
            nc.scalar.activation(
                sp[:], d2_ps[:], Sqrt,
                bias=qq_cols[hh][:, j : j + 1], scale=-1.0 / 32.0,
            )
            spos[(hh, j)] = sp

        score_sb = score_p.tile([128, NJ * L], F32, tag="score")
        for j in range(NJ):
            nc.vector.tensor_scalar_mul(
                score_sb[:, j * L : (j + 1) * L],
                spos[(hh, j)][:],
                aux_t[:, 6 + j : 7 + j],
            )
        nc.sync.dma_start(
            score_d[hh].rearrange("(j p) k -> p j k", p=128),
            score_sb[:].rearrange("p (j k) -> p j k", j=NJ),
        )

    # --- exp (all after all sqrts: single ACT table switch each way) -----
    dens = []
    e_tiles = {}
    for hh in range(HPC):
        den = den_p.tile([128, NJ], F32, tag="den")
        dens.append(den)
        for j in range(NJ):
            e = e_p.tile([128, L], F32, tag="e")
            nc.scalar.activation(
                e[:], spos[(hh, j)][:], Exp, scale=-1.0,
                accum_out=den[:, j : j + 1],
            )
            e_tiles[(hh, j)] = e

    # --- A.V: transpose e, matmul with V natural, untranspose, scale -----
    out_sb = outsb_p.tile([128, NJ * W], F32, tag="outsb")
    for hh in range(HPC):
        m, s = hh // 2, hh % 2
        et_sbs = []
        for i in range(NJ):
            et_ps = psA.tile([128, L], F32, tag="psA")
            for j in range(NJ):
                nc.tensor.transpose(
                    et_ps[:, j * 128 : (j + 1) * 128],
                    e_tiles[(hh, j)][:, i * 128 : (i + 1) * 128],
                    ident[:],
                )
            et_sb = etsb_p.tile([128, L], F32, tag="etsb")
            nc.vector.tensor_copy(et_sb[:], et_ps[:])
            et_sbs.append(et_sb)
        av_ps = psB.tile([64, L], F32, tag="psB")
        for i in range(NJ):
            nc.tensor.matmul(
                av_ps[:],
                vnat[i][:, hh * 64 : (hh + 1) * 64],
                et_sbs[i][:],
                start=(i == 0),
                stop=(i == 3),
            )
        av_sb = avsb_p.tile([64, L], F32, tag="avsb")
        nc.vector.tensor_copy(av_sb[:], av_ps[:])
        tr_ps = psB.tile([128, NJ * 64], F32, tag="psB")
        for j in range(NJ):
            nc.tensor.transpose(
                tr_ps[:, j * 64 : (j + 1) * 64],
                av_sb[0:64, j * 128 : (j + 1) * 128],
                ident[0:64, 0:64],
            )
        scale = den_p.tile([128, NJ], F32, tag="scale")
        nc.vector.reciprocal(scale[:], dens[hh][:])
        nc.vector.tensor_mul(scale[:], scale[:], aux_t[:, 10:14])
        for j in range(NJ):
            nc.vector.tensor_scalar_mul(
                out_sb[:, j * W + hh * 64 : j * W + (hh + 1) * 64],
                tr_ps[:, j * 64 : (j + 1) * 64],
                scale[:, j : j + 1],
            )
    nc.sync.dma_start(
        out_d.rearrange("(j p) n -> p j n", p=128),
        out_sb[:].rearrange("p (j n) -> p j n", j=NJ),
    )


_NC_CACHE = None


def build():
    global _NC_CACHE
    if _NC_CACHE is not None:
        return _NC_CACHE
    nc = bacc.Bacc("TRN2", target_bir_lowering=False, debug=False,
                   num_devices=N_CORES)
    xq = nc.dram_tensor("xq", [U, L], F32, kind="ExternalInput").ap()
    xk = nc.dram_tensor("xk", [U, L], F32, kind="ExternalInput").ap()
    xv = nc.dram_tensor("xv", [U, L], F32, kind="ExternalInput").ap()
    wq = nc.dram_tensor("wq", [U, W], F32, kind="ExternalInput").ap()
    wk = nc.dram_tensor("wk", [U, W], F32, kind="ExternalInput").ap()
    wv = nc.dram_tensor("wv", [U, W], F32, kind="ExternalInput").ap()
    aux = nc.dram_tensor("aux", [128, 16], F32, kind="ExternalInput").ap()
    out_d = nc.dram_tensor("out_d", [L, W], F32, kind="ExternalOutput").ap()
    score_d = nc.dram_tensor("score_d", [HPC, L, L], F32, kind="ExternalOutput").ap()
    with tile.TileContext(nc) as tc, ExitStack() as ctx:
        emit(ctx, tc, xq, xk, xv, wq, wk, wv, aux, out_d, score_d)
    nc.compile()
    _NC_CACHE = nc
    return nc


def make_in_maps(queries, keys, values, Wq, bq, Wk, bk, Wv, bv):
    """Build the per-core input dicts (host-side sharding/preprocessing)."""
    queries = np.asarray(queries, np.float32)
    keys = np.asarray(keys, np.float32)
    values = np.asarray(values, np.float32)
    Wq = np.asarray(Wq, np.float32)
    Wk = np.asarray(Wk, np.float32)
    Wv = np.asarray(Wv, np.float32)
    bq = np.asarray(bq, np.float32)
    bk = np.asarray(bk, np.float32)
    bv = np.asarray(bv, np.float32)

    qm = np.sign(np.abs(queries.sum(-1)))        # [B, L]
    in_maps = []
    for c in range(N_CORES):
        g, b = c % NG, c // NG
        sl = slice(g * W, (g + 1) * W)
        aux = np.zeros((128, 16), np.float32)
        for m in range(2):
            aux[:, 0 + m] = bq[sl][m * 128 : (m + 1) * 128]
            aux[:, 2 + m] = bk[sl][m * 128 : (m + 1) * 128]
            aux[:, 4 + m] = bv[sl][m * 128 : (m + 1) * 128]
        qmb = qm[b]                               # [L]
        for j in range(NJ):
            aux[:, 6 + j] = -qmb[j * 128 : (j + 1) * 128]
            aux[:, 10 + j] = qmb[j * 128 : (j + 1) * 128]
        in_maps.append({
            "xq": np.ascontiguousarray(queries[b].T),
            "xk": np.ascontiguousarray(keys[b].T),
            "xv": np.ascontiguousarray(values[b].T),
            "wq": np.ascontiguousarray(Wq[:, sl]),
            "wk": np.ascontiguousarray(Wk[:, sl]),
            "wv": np.ascontiguousarray(Wv[:, sl]),
            "aux": aux,
        })
    return in_maps


def gather(results):
    """Assemble per-core outputs into full (out, attn_score)."""
    out = np.empty((B, L, U), np.float32)
    attn_score = np.empty((H * B, L, L), np.float32)
    for c in range(N_CORES):
        g, b = c % NG, c // NG
        out[b, :, g * W : (g + 1) * W] = results[c]["out_d"]
        sd = results[c]["score_d"]
        for hh in range(HPC):
            attn_score[(g * HPC + hh) * B + b] = sd[hh]
    return out, attn_score


def kernel(queries, keys, values, Wq, bq, Wk, bk, Wv, bv):
    nc = build()
    in_maps = make_in_maps(queries, keys, values, Wq, bq, Wk, bk, Wv, bv)
    res = run_bass_kernel_spmd(nc, in_maps, core_ids=list(range(N_CORES)))
    return gather(res.results)


# revision 37
# speedup vs baseline: 1438.6864x; 1438.6864x over previous
"""Trainium2 Bass kernel for nn_DiffAttn (8-head euclidean-distance attention).

Sharding over 8 NeuronCores: core c = (head-group g = c % 2, batch b = c // 2).
Each core projects its batch's 512 tokens through its 4 heads' weight slices
(Q/K/V column slices), computes 4 attention units (one per head) and writes
a dense [512, 256] output slice plus a dense [4, 512, 512] score slice.

Math per core (matmuls bf16 with fp32 PSUM accumulation, softmax math fp32):
  QTX/KTX_h  = relu(W_h.T @ xT + b) in augmented [97, 512] tiles:
               rows 0-63 activations, row 64 ones(Q) / -kk_hi/2 (K),
               rows 65-95 zeros, row 96 ones(Q) / -kk_lo/2 (K);
               the kk hi/lo bf16 split keeps d2 near fp32-exact.
  psum       = QTX_h.T @ KTX_h            (one K=97 matmul = qk - kk/2)
  s_pos      = sqrt(psum*(-1/32)+qq/64)   (ACT; qq rides the bias port)
  score_out  = s_pos * (-qm)              (DVE per-partition scalar)
  e          = exp(-s_pos) -> bf16        (ACT), denom = DVE reduce
  eT, V_nat  = xbar DMA transposes        (stripe cancels in contraction)
  out        = transpose(V_nat_slice.T @ eT) * (qm/denom)

Scheduling notes (engines are in-order; this drove most of the layout):
  - sqrt and exp live in different ACT table sets (1283ns reload per
    switch); exps carry explicit deps on the half-batch's last sqrt.
  - bookkeeping is split DVE (m=0 heads) / ACT (m=1 heads) so the Q/K
    relu+square chains run in parallel.
  - score stores go through SWDGE (gpsimd) so the SP HWDGE ring serves
    input loads and xbar transposes without queueing behind 1MB stores.
"""
import sys
sys.path.insert(0, '/opt/trn_rl_repo')

import numpy as np
import ml_dtypes
from contextlib import ExitStack

import concourse.bass as bass
import concourse.bacc as bacc
import concourse.tile as tile
from concourse import mybir, masks
from concourse.bass_utils import run_bass_kernel_spmd
from concourse.tile import add_dep_helper

F32 = mybir.dt.float32
BF16 = mybir.dt.bfloat16
NP_BF16 = ml_dtypes.bfloat16
H, B, L, U = 8, 4, 512, 512
D = U // H              # 64
N_CORES = 8
HPC = 4                 # heads per core
NG = H // HPC           # head groups = 2
W = HPC * D             # per-core output width = 256
NJ = L // 128           # 4 lq tiles per unit


def emit(ctx: ExitStack, tc, xq, xk, xv, wq, wk, wv, aux, out_d, score_d):
    nc = tc.nc

    aux_p = ctx.enter_context(tc.tile_pool(name="aux", bufs=1))
    const_p = ctx.enter_context(tc.tile_pool(name="const", bufs=1))
    xw_p = ctx.enter_context(tc.tile_pool(name="xw", bufs=3))
    qkv_p = ctx.enter_context(tc.tile_pool(name="qkv", bufs=2))
    qx_p = ctx.enter_context(tc.tile_pool(name="qx", bufs=4))
    kx_p = ctx.enter_context(tc.tile_pool(name="kx", bufs=4))
    vnat_p = ctx.enter_context(tc.tile_pool(name="vnat", bufs=2))
    sq_p = ctx.enter_context(tc.tile_pool(name="sq", bufs=8))
    cols_p = ctx.enter_context(tc.tile_pool(name="cols", bufs=4))
    spos_p = ctx.enter_context(tc.tile_pool(name="spos", bufs=4))
    score_p = ctx.enter_context(tc.tile_pool(name="score", bufs=2))
    e_p = ctx.enter_context(tc.tile_pool(name="e", bufs=4))
    etsb_p = ctx.enter_context(tc.tile_pool(name="etsb", bufs=2))
    den_p = ctx.enter_context(tc.tile_pool(name="den", bufs=4))
    avsb_p = ctx.enter_context(tc.tile_pool(name="avsb", bufs=2))
    outsb_p = ctx.enter_context(tc.tile_pool(name="outsb", bufs=1))
    psA = ctx.enter_context(tc.tile_pool(name="psA", bufs=4, space="PSUM"))
    psB = ctx.enter_context(tc.tile_pool(name="psB", bufs=3, space="PSUM"))

    Sqrt = mybir.ActivationFunctionType.Sqrt
    Exp = mybir.ActivationFunctionType.Exp
    Relu = mybir.ActivationFunctionType.Relu

    # --- input staging first so the SP DMA ring starts immediately ------
    aux_t = aux_p.tile([128, 16], F32, tag="aux")
    nc.sync.dma_start(aux_t[:], aux)
    x_sb = {}
    w_sb = {}
    for name, xd, wd in (("k", xk, wk), ("q", xq, wq), ("v", xv, wv)):
        wt = xw_p.tile([128, 4 * W], BF16, tag="w", name=f"w_{name}")
        nc.sync.dma_start(
            wt[:].rearrange("p (c n) -> p c n", c=4),
            wd.rearrange("(c p) n -> p c n", p=128),
        )
        w_sb[name] = wt
        xt = xw_p.tile([128, 4 * L], BF16, tag="x", name=f"x_{name}")
        xr = xd.rearrange("(c p) l -> c p l", p=128)
        for c in range(4):
            nc.sync.dma_start(xt[:, c * L : (c + 1) * L], xr[c])
        x_sb[name] = xt

    ident_b = const_p.tile([128, 128], BF16, tag="identb")
    masks.make_identity(nc, ident_b[:])
    ident_f = const_p.tile([128, 128], F32, tag="identf")
    masks.make_identity(nc, ident_f[:])
    ones_b = const_p.tile([128, 128], BF16, tag="onesb")
    nc.gpsimd.memset(ones_b[:], 1.0)
    mhalf_col = const_p.tile([128, 1], BF16, tag="mhalfcol")
    nc.gpsimd.memset(mhalf_col[:], -0.5)

    # --- projections -----------------------------------------------------
    # aux columns: 0-1 bq, 2-3 bk, 4-5 bv, 6-9 qm_neg, 10-13 qm_pos
    # Q/K land in per-head augmented tiles [97, 512]:
    #   rows 0-63 = head activations, row 64 = ones (Q) / -kk_hi/2 (K),
    #   rows 65-95 = zeros, row 96 = ones (Q) / -kk_lo/2 (K).
    # One K=97 matmul then computes qk - kk/2 without rank-1 updates.
    qtx = [qx_p.tile([97, L], BF16, tag="qtx", name=f"qtx{hh}")
           for hh in range(HPC)]
    ktx = [kx_p.tile([97, L], BF16, tag="ktx", name=f"ktx{hh}")
           for hh in range(HPC)]
    for hh in range(HPC):
        nc.gpsimd.memset(qtx[hh][64:97, :], 0.0)
        nc.gpsimd.memset(qtx[hh][64:65, :], 1.0)
        nc.gpsimd.memset(qtx[hh][96:97, :], 1.0)
        nc.gpsimd.memset(ktx[hh][64:97, :], 0.0)

    vt_tiles = []
    sq_tiles = {}
    Square = mybir.ActivationFunctionType.Square
    for name, boff in (("k", 2), ("q", 0)):
        xt, wt = x_sb[name], w_sb[name]
        for m in range(2):
            pp = psA.tile([128, L], F32, tag="psA", name=f"pp_{name}{m}")
            for c in range(4):
                nc.tensor.matmul(
                    pp[:],
                    wt[:, c * W + m * 128 : c * W + (m + 1) * 128],
                    xt[:, c * L : (c + 1) * L],
                    start=(c == 0),
                    stop=(c == 3),
                )
            for s in range(2):
                hh = 2 * m + s
                bias = aux_t[s * 64 : (s + 1) * 64, boff + m : boff + m + 1]
                sq = sq_p.tile([64, L], BF16, tag="sq", name=f"sq_{name}{hh}")
                dst = ktx[hh] if name == "k" else qtx[hh]
                if m == 0:
                    # heads 0-1 bookkeeping on DVE
                    nc.vector.tensor_scalar(
                        dst[0:64, :],
                        pp[s * 64 : (s + 1) * 64, :],
                        bias, 0.0,
                        mybir.AluOpType.add, mybir.AluOpType.max,
                    )
                    nc.vector.tensor_mul(sq[:], dst[0:64, :], dst[0:64, :])
                else:
                    # heads 2-3 bookkeeping on ACT (Square is in every set)
                    nc.scalar.activation(
                        dst[0:64, :], pp[s * 64 : (s + 1) * 64, :],
                        Relu, bias=bias, scale=1.0,
                    )
                    nc.scalar.activation(sq[:], dst[0:64, :], Square)
                sq_tiles[(name, hh)] = sq

    # --- V projection + natural layout (needed only by A.V) -------------
    xt, wt = x_sb["v"], w_sb["v"]
    for m in range(2):
        pp = psA.tile([128, L], F32, tag="psA", name=f"pp_v{m}")
        for c in range(4):
            nc.tensor.matmul(
                pp[:],
                wt[:, c * W + m * 128 : c * W + (m + 1) * 128],
                xt[:, c * L : (c + 1) * L],
                start=(c == 0),
                stop=(c == 3),
            )
        t = qkv_p.tile([128, L], BF16, tag="vt", name=f"vt{m}")
        nc.vector.tensor_scalar(
            t[:], pp[:], aux_t[:, 4 + m : 5 + m], 0.0,
            mybir.AluOpType.add, mybir.AluOpType.max,
        )
        vt_tiles.append(t)
    vnat = []
    for m in range(2):
        vm = vnat_p.tile([128, NJ * 128], BF16, tag="vnat", name=f"vnatm{m}")
        nc.sync.dma_start(
            vm[:].rearrange("p (i c) -> p i c", i=NJ),
            vt_tiles[m][:],
            transpose=True,
        )
        vnat.append(vm)

    # --- qq columns and kk rows for all heads (before any d2 so heads
    #     don't serialize through the in-order PE/DVE queues) --------------
    for hh in range(HPC):
        krow_ps = psB.tile([1, L], F32, tag="psB", name=f"krow_ps{hh}")
        nc.tensor.matmul(
            krow_ps[0:1, :],
            mhalf_col[0:64, 0:1],
            sq_tiles[("k", hh)][0:64, :],
            start=True,
            stop=True,
        )
        nc.vector.tensor_copy(ktx[hh][64:65, :], krow_ps[0:1, :])
        nc.vector.tensor_sub(
            ktx[hh][96:97, :], krow_ps[0:1, :], ktx[hh][64:65, :])

    qq_cols = []
    for hh in range(HPC):
        colT_ps = psB.tile([128, NJ], F32, tag="psB", name=f"colT_ps{hh}")
        for j in range(NJ):
            nc.tensor.matmul(
                colT_ps[:, j : j + 1],
                sq_tiles[("q", hh)][0:64, j * 128 : (j + 1) * 128],
                ones_b[0:64, 0:1],
                start=True,
                stop=True,
            )
        qq_col = cols_p.tile([128, NJ], F32, tag="qqcol",
                             name=f"qq_col{hh}")
        nc.vector.tensor_scalar_mul(qq_col[:], colT_ps[:], 1.0 / 64.0)
        qq_cols.append(qq_col)
    # --- distance matmul + sqrt + exp, in half-batches -------------------
    # sqrt and exp live in different ACT tables; batching pairs of heads
    # costs one extra table switch but lets the A.V tail of the first pair
    # overlap the second pair's sqrts.
    spos = [spos_p.tile([128, NJ * L], F32, tag="spos", name=f"spos_{hh}")
            for hh in range(HPC)]
    dens = []
    e_tiles = []
    for hh in range(HPC):
        den = den_p.tile([128, NJ], F32, tag="den", name=f"den{hh}")
        dens.append(den)
        e = e_p.tile([128, NJ * L], BF16, tag="e", name=f"e_{hh}")
        e_tiles.append(e)
    for half in range(2):
        heads = (2 * half, 2 * half + 1)
        last_sqrt = None
        for hh in heads:
            for j in range(NJ):
                d2_ps = psA.tile([128, L], F32, tag="psA",
                                 name=f"d2_{hh}{j}")
                nc.tensor.matmul(
                    d2_ps[:],
                    qtx[hh][0:97, j * 128 : (j + 1) * 128],
                    ktx[hh][0:97, :],
                    start=True,
                    stop=True,
                )
                last_sqrt = nc.scalar.activation(
                    spos[hh][:, j * L : (j + 1) * L], d2_ps[:], Sqrt,
                    bias=qq_cols[hh][:, j : j + 1],
                    scale=-1.0 / 32.0,
                )
        for hh in heads:
            for j in range(NJ):
                ei = nc.scalar.activation(
                    e_tiles[hh][:, j * L : (j + 1) * L],
                    spos[hh][:, j * L : (j + 1) * L], Exp, scale=-1.0,
                )
                add_dep_helper(ei.ins, last_sqrt.ins, sync=False,
                               reason="exp after half-batch sqrts")

    # --- score outputs (off the DVE critical chain: emitted after all
    #     per-head qq/kk row work so heads don't serialize through them) ---
    for hh in range(HPC):
        score_sb = score_p.tile([128, NJ * L], F32, tag="score",
                                name=f"score_sb{hh}")
        for j in range(NJ):
            nc.vector.tensor_scalar_mul(
                score_sb[:, j * L : (j + 1) * L],
                spos[hh][:, j * L : (j + 1) * L],
                aux_t[:, 6 + j : 7 + j],
            )
        nc.gpsimd.dma_start(
            score_d[hh].rearrange("(j p) k -> p j k", p=128),
            score_sb[:].rearrange("p (j k) -> p j k", j=NJ),
        )

    # --- A.V: xbar-transpose e per head, 4 accumulating matmuls ---------
    # et[p, (j*NJ+i)*128 + c] = e_h[c, j*L + i*128 + p]; for fixed i a 3D
    # rhs AP [128, j, c] streams all 512 lq columns in one matmul.
    out_sb = outsb_p.tile([128, NJ * W], F32, tag="outsb")
    for hh in range(HPC):
        m, s = hh // 2, hh % 2
        et_sb = etsb_p.tile([128, NJ * L], BF16, tag="etsb",
                            name=f"et_sb{hh}")
        nc.sync.dma_start(
            et_sb[:].rearrange("p (blk c) -> p blk c", c=128),
            e_tiles[hh][:],
            transpose=True,
        )
        et4 = et_sb[:].rearrange("p (j i c) -> p j i c", i=NJ, c=128)
        av_ps = psB.tile([64, L], F32, tag="psB", name=f"av_ps{hh}")
        for i in range(NJ):
            nc.tensor.matmul(
                av_ps[:].rearrange("p (j c) -> p j c", c=128),
                vnat[m][:, i * 128 + s * 64 : i * 128 + (s + 1) * 64],
                et4[:, :, i, :],
                start=(i == 0),
                stop=(i == 3),
            )
        for j in range(NJ):
            nc.vector.reduce_sum(
                dens[hh][:, j : j + 1],
                e_tiles[hh][:, j * L : (j + 1) * L],
                axis=mybir.AxisListType.X,
            )
        av_sb = avsb_p.tile([64, L], F32, tag="avsb", name=f"av_sb{hh}")
        nc.vector.tensor_copy(av_sb[:], av_ps[:])
        tr_ps = psB.tile([128, NJ * 64], F32, tag="psB", name=f"tr_ps{hh}")
        for j in range(NJ):
            nc.tensor.transpose(
                tr_ps[:, j * 64 : (j + 1) * 64],
                av_sb[0:64, j * 128 : (j + 1) * 128],
                ident_f[0:64, 0:64],
            )
        scale = den_p.tile([128, NJ], F32, tag="scale", name=f"scale{hh}")
        nc.vector.reciprocal(scale[:], dens[hh][:])
        nc.vector.tensor_mul(scale[:], scale[:], aux_t[:, 10:14])
        for j in range(NJ):
            nc.vector.tensor_scalar_mul(
                out_sb[:, j * W + hh * 64 : j * W + (hh + 1) * 64],
                tr_ps[:, j * 64 : (j + 1) * 64],
                scale[:, j : j + 1],
            )
    nc.sync.dma_start(
        out_d.rearrange("(j p) n -> p j n", p=128),
        out_sb[:].rearrange("p (j n) -> p j n", j=NJ),
    )


_NC_CACHE = None


def build():
    global _NC_CACHE
    if _NC_CACHE is not None:
        return _NC_CACHE
    nc = bacc.Bacc("TRN2", target_bir_lowering=False, debug=False,
                   num_devices=N_CORES)
    xq = nc.dram_tensor("xq", [U, L], BF16, kind="ExternalInput").ap()
    xk = nc.dram_tensor("xk", [U, L], BF16, kind="ExternalInput").ap()
    xv = nc.dram_tensor("xv", [U, L], BF16, kind="ExternalInput").ap()
    wq = nc.dram_tensor("wq", [U, W], BF16, kind="ExternalInput").ap()
    wk = nc.dram_tensor("wk", [U, W], BF16, kind="ExternalInput").ap()
    wv = nc.dram_tensor("wv", [U, W], BF16, kind="ExternalInput").ap()
    aux = nc.dram_tensor("aux", [128, 16], F32, kind="ExternalInput").ap()
    out_d = nc.dram_tensor("out_d", [L, W], F32, kind="ExternalOutput").ap()
    score_d = nc.dram_tensor("score_d", [HPC, L, L], F32,
                             kind="ExternalOutput").ap()
    with tile.TileContext(nc) as tc, ExitStack() as ctx:
        emit(ctx, tc, xq, xk, xv, wq, wk, wv, aux, out_d, score_d)
    nc.compile()
    _NC_CACHE = nc
    return nc


def make_in_maps(queries, keys, values, Wq, bq, Wk, bk, Wv, bv):
    """Build the per-core input dicts (host-side sharding/preprocessing)."""
    queries = np.asarray(queries, np.float32)
    keys = np.asarray(keys, np.float32)
    values = np.asarray(values, np.float32)
    Wq = np.asarray(Wq, np.float32)
    Wk = np.asarray(Wk, np.float32)
    Wv = np.asarray(Wv, np.float32)
    bq = np.asarray(bq, np.float32)
    bk = np.asarray(bk, np.float32)
    bv = np.asarray(bv, np.float32)

    qm = np.sign(np.abs(queries.sum(-1)))        # [B, L]
    in_maps = []
    for c in range(N_CORES):
        g, b = c % NG, c // NG
        sl = slice(g * W, (g + 1) * W)
        aux = np.zeros((128, 16), np.float32)
        for m in range(2):
            aux[:, 0 + m] = bq[sl][m * 128 : (m + 1) * 128]
            aux[:, 2 + m] = bk[sl][m * 128 : (m + 1) * 128]
            aux[:, 4 + m] = bv[sl][m * 128 : (m + 1) * 128]
        qmb = qm[b]                               # [L]
        for j in range(NJ):
            aux[:, 6 + j] = -qmb[j * 128 : (j + 1) * 128]
            aux[:, 10 + j] = qmb[j * 128 : (j + 1) * 128]
        in_maps.append({
            "xq": np.ascontiguousarray(queries[b].T).astype(NP_BF16),
            "xk": np.ascontiguousarray(keys[b].T).astype(NP_BF16),
            "xv": np.ascontiguousarray(values[b].T).astype(NP_BF16),
            "wq": np.ascontiguousarray(Wq[:, sl]).astype(NP_BF16),
            "wk": np.ascontiguousarray(Wk[:, sl]).astype(NP_BF16),
            "wv": np.ascontiguousarray(Wv[:, sl]).astype(NP_BF16),
            "aux": aux,
        })
    return in_maps


def gather(results):
    """Assemble per-core outputs into full (out, attn_score)."""
    out = np.empty((B, L, U), np.float32)
    attn_score = np.empty((H * B, L, L), np.float32)
    for c in range(N_CORES):
        g, b = c % NG, c // NG
        out[b, :, g * W : (g + 1) * W] = results[c]["out_d"]
        sd = results[c]["score_d"]
        for hh in range(HPC):
            attn_score[(g * HPC + hh) * B + b] = sd[hh]
    return out, attn_score


def kernel(queries, keys, values, Wq, bq, Wk, bk, Wv, bv):
    nc = build()
    in_maps = make_in_maps(queries, keys, values, Wq, bq, Wk, bk, Wv, bv)
    res = run_bass_kernel_spmd(nc, in_maps, core_ids=list(range(N_CORES)))
    return gather(res.results)
